# revision 1
# baseline (speedup 1.0000x reference)
"""Trainium2 Bass kernel: binarized-MLP forward (784-256-128-32-10, ste_sign).

Strategy
--------
Pure data parallel over 8 NeuronCores: batch 65536 -> 8 shards of 8192 rows;
the tiny sign-binarized weights are replicated (binarized + transposed on the
host). Each core runs the full 4-layer network on its shard; outputs are
gathered on the host. No collectives needed (forward only).

On-chip the network runs feature-major: activations live as [features, batch]
tiles and every matmul streams batch as the moving dimension, so layer N's
output feeds layer N+1 with no transposes between layers. x is pre-transposed
to [784, B] on the host so the contraction dim lands on the partition axis
straight out of DMA.

Layer 1 (x is real-valued fp32; everything downstream only sees sign(h1)) uses
a Dekker-style two-pass fp16 split: the host ships xh = fp16(x) plus the
fp16-rounded residual (x - xh), and both passes (weights +-1, exact in fp16)
accumulate into one PSUM group at 1 cycle/row each (vs 4 for native fp32).
This recovers ~21 mantissa bits — beyond the fp32 reference's own rounding
noise (measured on HW: max err 2.1e-5 on 784-length dots, 0 sign flips in
65536) — at half the PE cost of fp32 and 4 bytes/element of DMA (fp32's
bandwidth) instead of 6 for an f32r+bf16 split.

Layers 2-4 have +-1 inputs and +-1 weights, so bf16 matmuls are exact
(integer partial sums <= 256); the sign activations run on the scalar engine.
ACT Sign(0) = 0 on this HW, so integer-valued pre-activations (layers 2,3) use
Sign(h + 0.5), which reproduces the reference's sign(0)=+1 exactly. The final
logits are integers in [-32, 32], computed exactly.

x is loaded in [k-tile, 1024]-column super-tiles (fewer, larger DMAs — the
DMA queue is the second-busiest resource), split across both HWDGE engines
(xq on SP, residual on ACT), with the first super-group prefetched ahead of
the weight loads so the PE starts early.

This walrus build rejects instructions carrying more than one semaphore wait
("Too many sync wait commands"), so after Tile scheduling, excess waits are
split onto preceding same-engine NoOps (fix_sync_waits).
"""
import sys
sys.path.insert(0, '/opt/trn_rl_repo')
import numpy as np
import ml_dtypes
import concourse.bass as bass
import concourse.mybir as mybir
from concourse import tile
from concourse.bass_utils import run_bass_kernel_spmd

BF16 = ml_dtypes.bfloat16
F32 = mybir.dt.float32
FP16 = mybir.dt.float16
BF = mybir.dt.bfloat16
AF = mybir.ActivationFunctionType

N_CORES = 8
B_LOC = 8192          # batch rows per core
NB = 512              # batch columns per compute chunk (one fp32 PSUM bank)
NCHUNK = B_LOC // NB
NB_LOAD = 1024        # batch columns per x DMA super-tile
K1 = 784
KTILES = [(k, min(128, K1 - k)) for k in range(0, K1, 128)]  # 6x128 + 16
F1, F2, F3, F4 = 256, 128, 32, 10
MAX_WAITS = 1
PASS_DT = ((FP16, np.float16), (FP16, np.float16))  # L1: fp16 hi, fp16 residual


def fix_sync_waits(nc):
    for fn in nc.m.functions:
        for bb in fn.blocks:
            out = []
            changed = False
            for ins in bb.instructions:
                si = ins.sync_info
                waits = list(si.on_wait) if si is not None else []
                if len(waits) > MAX_WAITS:
                    head, keep = waits[:-MAX_WAITS], waits[-MAX_WAITS:]
                    k = 0
                    while head:
                        chunk, head = head[:MAX_WAITS], head[MAX_WAITS:]
                        nop = mybir.InstNoOp(
                            name=f"{ins.name}-wsplit{k}", engine=ins.engine)
                        nop.sync_info = mybir.SyncInfo(on_wait=chunk, on_update=[])
                        out.append(nop)
                        k += 1
                    ins.sync_info = mybir.SyncInfo(
                        on_wait=keep, on_update=list(si.on_update))
                    changed = True
                out.append(ins)
            if changed:
                bb.instructions = out


def round_mant11(a):
    """fp32 -> RNE at 11 explicit mantissa bits (= HW f32r input rounding)."""
    u = np.ascontiguousarray(a).view(np.uint32)
    drop = 12
    lsb = ((u >> drop) & 1).astype(np.uint32)
    r = ((u + np.uint32((1 << (drop - 1)) - 1) + lsb) >> drop) << drop
    return r.view(np.float32)


def build_nc(rep=1):
    nc = bass.Bass()
    x_d = nc.declare_dram_parameter("xqs", [K1, 2, B_LOC], FP16, isOutput=False)
    w1_d = nc.declare_dram_parameter("w1sT", [K1, F1], FP16, isOutput=False)
    w2_d = nc.declare_dram_parameter("w2sT", [F1, F2], BF, isOutput=False)
    w3_d = nc.declare_dram_parameter("w3sT", [F2, F3], BF, isOutput=False)
    w4_d = nc.declare_dram_parameter("w4sT", [F3, F4], BF, isOutput=False)
    out_d = nc.declare_dram_parameter("out", [F4, B_LOC], F32, isOutput=True)

    with tile.TileContext(nc) as tc:
        with tc.tile_pool(name="wpool", bufs=1) as wpool, \
             tc.tile_pool(name="xtpool", bufs=2) as xtpool, \
             tc.tile_pool(name="apool", bufs=2) as apool, \
             tc.tile_pool(name="opool", bufs=2) as opool, \
             tc.tile_pool(name="ps1", bufs=2, space="PSUM") as ps1, \
             tc.tile_pool(name="ps2", bufs=2, space="PSUM") as ps2, \
             tc.tile_pool(name="ps34", bufs=1, space="PSUM") as ps34:
            # head: interleave weight k-tiles with the first x super-group so
            # the first matmul's operands land back-to-back. Full k-tiles load
            # hi|res combined in ONE DMA ([kw, 2*NB_LOAD]); the 16-row k-tail
            # loads hi into partitions 0:16 and res into 16:32 of one [32,*]
            # tile so both passes' tails run as a single 32-contraction matmul.
            NKF = len(KTILES) - 1           # full 128-row k-tiles
            TK0, TKW = KTILES[-1]           # 768, 16
            w1_t = [None] * NKF
            w1tail = None
            xg0 = []

            def load_group(g, r):
                gb0 = (g % (B_LOC // NB_LOAD)) * NB_LOAD
                row = []
                for i, (k0, kw) in enumerate(KTILES[:NKF]):
                    t = xtpool.tile([kw, 2 * NB_LOAD], FP16,
                                    name=f"xG_{r}_{g}_{i}", tag=f"xG{i}")
                    eng = nc.sync if (g + i) % 2 == 0 else nc.scalar
                    eng.dma_start(t[:], x_d[k0:k0 + kw, :, gb0:gb0 + NB_LOAD])
                    row.append(t)
                tt = xtpool.tile([32, NB_LOAD], FP16,
                                 name=f"xGt_{r}_{g}", tag="xGt")
                eng = nc.sync if g % 2 == 0 else nc.scalar
                eng.dma_start(tt[:TKW, :], x_d[TK0:TK0 + TKW, 0, gb0:gb0 + NB_LOAD])
                eng.dma_start(tt[16:16 + TKW, :],
                              x_d[TK0:TK0 + TKW, 1, gb0:gb0 + NB_LOAD])
                row.append(tt)
                return row

            # head: pair each weight k-tile with its group-0 x tile, spread
            # across both HWDGE queues so the accumulation chain's operands
            # land in matmul order.
            gb0 = 0
            for i, (k0, kw) in enumerate(KTILES[:NKF]):
                eng = nc.sync if i % 2 == 0 else nc.scalar
                t = wpool.tile([kw, F1], FP16, name=f"w1t_{i}")
                eng.dma_start(t[:], w1_d[k0:k0 + kw, :])
                w1_t[i] = t
                tx = xtpool.tile([kw, 2 * NB_LOAD], FP16,
                                 name=f"xG_0_0_{i}", tag=f"xG{i}")
                eng.dma_start(tx[:], x_d[k0:k0 + kw, :, 0:NB_LOAD])
                xg0.append(tx)
            w1tail = wpool.tile([32, F1], FP16, name="w1tail")
            nc.sync.dma_start(w1tail[:TKW, :], w1_d[TK0:TK0 + TKW, :])
            nc.scalar.dma_start(w1tail[16:16 + TKW, :], w1_d[TK0:TK0 + TKW, :])
            tt0 = xtpool.tile([32, NB_LOAD], FP16, name="xGt_0_0", tag="xGt")
            nc.sync.dma_start(tt0[:TKW, :], x_d[TK0:TK0 + TKW, 0, 0:NB_LOAD])
            nc.scalar.dma_start(tt0[16:16 + TKW, :],
                                x_d[TK0:TK0 + TKW, 1, 0:NB_LOAD])
            xg0.append(tt0)
            w2_t = []
            for i in range(2):
                t = wpool.tile([128, F2], BF, name=f"w2t{i}")
                nc.scalar.dma_start(t[:], w2_d[i * 128:(i + 1) * 128, :])
                w2_t.append(t)
            w3_t = wpool.tile([F2, F3], BF, name="w3t")
            nc.scalar.dma_start(w3_t[:], w3_d[:, :])
            w4_t = wpool.tile([F3, F4], BF, name="w4t")
            nc.scalar.dma_start(w4_t[:], w4_d[:, :])
            zb = wpool.tile([128, 1], F32, name="zb")
            nc.vector.memset(zb[:], 0.0)
            hb = wpool.tile([128, 1], F32, name="hb")
            nc.vector.memset(hb[:], 0.5)

            nsub = NB_LOAD // NB
            xg = None
            for r in range(rep):
                for c in range(NCHUNK):
                    b0 = c * NB
                    g, j = divmod(c, nsub)
                    if j == 0:
                        xg = xg0 if (r == 0 and g == 0) else load_group(g, r)
                    # rhs slices for this 512-chunk: [hi | res] halves per tile
                    rhs0 = [t[:, j * NB:(j + 1) * NB] for t in xg[:NKF]]
                    rhs1 = [t[:, NB_LOAD + j * NB: NB_LOAD + (j + 1) * NB]
                            for t in xg[:NKF]]
                    rhs_tail = xg[NKF][:, j * NB:(j + 1) * NB]
                    a1 = []
                    for f in range(2):
                        p1 = ps1.tile([128, NB], F32, name=f"p1_{r}_{c}_{f}",
                                      tag="p1")
                        fs = slice(f * 128, (f + 1) * 128)
                        for i in range(NKF):
                            nc.tensor.matmul(p1[:], w1_t[i][:, fs], rhs0[i],
                                             start=(i == 0), stop=False)
                        for i in range(NKF):
                            nc.tensor.matmul(p1[:], w1_t[i][:, fs], rhs1[i],
                                             start=False, stop=False)
                        nc.tensor.matmul(p1[:], w1tail[:, fs], rhs_tail,
                                         start=False, stop=True)
                        s1 = apool.tile([128, NB], BF, name=f"a1_{r}_{c}_{f}",
                                        tag=f"a1{f}")
                        nc.scalar.activation(s1[:], p1[:], AF.Sign, bias=zb[:],
                                             scale=1.0)
                        a1.append(s1)
                    p2 = ps2.tile([F2, NB], F32, name=f"p2_{r}_{c}", tag="p2")
                    nc.tensor.matmul(p2[:], w2_t[0][:], a1[0][:], start=True,
                                     stop=False)
                    nc.tensor.matmul(p2[:], w2_t[1][:], a1[1][:], start=False,
                                     stop=True)
                    a2 = apool.tile([F2, NB], BF, name=f"a2_{r}_{c}", tag="a2")
                    nc.scalar.activation(a2[:], p2[:], AF.Sign, bias=hb[:],
                                         scale=1.0)
                    p3 = ps34.tile([F3, NB], F32, name=f"p3_{r}_{c}", tag="p3")
                    nc.tensor.matmul(p3[:], w3_t[:], a2[:], start=True, stop=True)
                    a3 = apool.tile([F3, NB], BF, name=f"a3_{r}_{c}", tag="a3")
                    nc.scalar.activation(a3[:], p3[:], AF.Sign, bias=hb[:F3, :],
                                         scale=1.0)
                    p4 = ps34.tile([F4, NB], F32, name=f"p4_{r}_{c}", tag="p4")
                    nc.tensor.matmul(p4[:], w4_t[:], a3[:], start=True, stop=True)
                    o = opool.tile([F4, NB], F32, name=f"o_{r}_{c}", tag="o")
                    nc.vector.tensor_copy(o[:], p4[:])
                    nc.sync.dma_start(out_d[:, b0:b0 + NB], o[:])
    fix_sync_waits(nc)
    return nc


def _sg(w):
    return np.where(w >= 0, np.float32(1.0), np.float32(-1.0))


_NC_CACHE = {}


def kernel(x, w1, w2, w3, w4):
    if "nc" not in _NC_CACHE:
        _NC_CACHE["nc"] = build_nc()
    nc = _NC_CACHE["nc"]

    x = np.ascontiguousarray(np.asarray(x).reshape(-1, K1), dtype=np.float32)
    w1sT = np.ascontiguousarray(_sg(np.asarray(w1)).T)
    wm = {
        "w1sT": w1sT.astype(np.float16),       # +-1 exact in fp16
        "w2sT": np.ascontiguousarray(_sg(np.asarray(w2)).T).astype(BF16),
        "w3sT": np.ascontiguousarray(_sg(np.asarray(w3)).T).astype(BF16),
        "w4sT": np.ascontiguousarray(_sg(np.asarray(w4)).T).astype(BF16),
    }
    xq = x.astype(np.float16)
    xs = (x - xq.astype(np.float32)).astype(np.float16)
    xqs = np.empty((K1, 2, x.shape[0]), np.float16)   # [784, 2, 65536]
    xqs[:, 0, :] = xq.T
    xqs[:, 1, :] = xs.T

    maps = []
    for c in range(N_CORES):
        m = dict(wm)
        m["xqs"] = xqs[:, :, c * B_LOC:(c + 1) * B_LOC]
        maps.append(m)

    res = None
    last_exc = None
    for attempt in range(3):
        try:
            res = run_bass_kernel_spmd(nc, maps, list(range(N_CORES)))
            break
        except Exception as e:  # transient NRT/device errors: retry
            last_exc = e
            import time
            time.sleep(5 * (attempt + 1))
    if res is None:
        raise last_exc
    outs = [r["out"] for r in res.results]                 # [10, 8192] each
    return np.ascontiguousarray(
        np.concatenate([o.T for o in outs], axis=0)).astype(np.float32)



# revision 13
# speedup vs baseline: 1.3689x; 1.3689x over previous
"""Trainium2 Bass kernel: binarized-MLP forward (784-256-128-32-10, ste_sign).

Strategy
--------
Pure data parallel over 8 NeuronCores: batch 65536 -> 8 shards of 8192 rows;
tiny sign-binarized weights replicated (binarized + packed on the host). Each
core runs the full 4-layer net on its shard feature-major ([features, batch]
tiles, batch as the moving dim) so layer N's output feeds layer N+1 with no
transposes. Outputs leave batch-major via a flipped final matmul.

Layer 1 (the only real-valued matmul; everything downstream only sees
sign(h1)) uses a multi-component split of x whose per-pass scales are folded
into the replicated weight k-rows, so all passes accumulate into ONE PSUM
group with no combine step:

  N8PASS=1: x ~= e4m3(x) + fp16(r1)            (3 B/elem, err ~2^-16|x|)
  N8PASS=2: x ~= e4m3(x) + e4m3(r1*16)/16 + fp16(r2)   (4 B/elem, ~2^-20|x|)

fp8 passes run as DoubleRow matmuls (2 k-tiles per instruction, 0.5
cycles/col -> 4x bf16 throughput); 784 = 98*8 for the fp8 part (4 DoubleRow
pairs per pass, no tail) and 112*7 for the fp16 part (7 full matmuls, no
tail). Layers 2-3 have +-1 inputs and +-1 weights, exact in fp8: activations
are written with feature-halves as DoubleRow slots ([128,2,512] / [64,2,512])
so L2/L3 are single DoubleRow units. L4 is flipped (lhsT = a3 slices, rhs =
w4^T moving, 10-wide): 10 cycles per matmul and batch-major [128,10] PSUM
output, staged into one [128, 640] SBUF tile and DMA'd out in 4 bulk
transfers (partition-major, 2560 B contiguous per partition).

HW Sign(0) = 0, so the integer-valued pre-activations (layers 2,3) use
Sign(h + 0.5), which reproduces the reference's sign(0)=+1 exactly.

This walrus build rejects instructions carrying more than one semaphore wait
("Too many sync wait commands"), so after Tile scheduling, excess waits are
split onto preceding same-engine NoOps (fix_sync_waits).
"""
import sys
sys.path.insert(0, '/opt/trn_rl_repo')
import numpy as np
import ml_dtypes
import concourse.bass as bass
import concourse.mybir as mybir
from concourse import tile
from concourse.bass_utils import run_bass_kernel_spmd

BF16 = ml_dtypes.bfloat16
E4M3 = ml_dtypes.float8_e4m3
F32 = mybir.dt.float32
FP16 = mybir.dt.float16
FP8 = mybir.dt.float8e4
BF = mybir.dt.bfloat16
AF = mybir.ActivationFunctionType
DR = mybir.MatmulPerfMode.DoubleRow

N_CORES = 8
B_LOC = 8192          # batch rows per core
NB = 512              # batch columns per compute chunk (one fp32 PSUM bank)
NCHUNK = B_LOC // NB
N8PASS = 1            # fp8 components of x (1 => +fp16 residual = 3 B/elem)
NT8 = 8 * N8PASS      # fp8 k-tiles of 98 rows
F1, F2, F3, F4 = 256, 128, 32, 10
MAX_WAITS = 1


def fix_sync_waits(nc):
    for fn in nc.m.functions:
        for bb in fn.blocks:
            out = []
            changed = False
            for ins in bb.instructions:
                si = ins.sync_info
                waits = list(si.on_wait) if si is not None else []
                if len(waits) > MAX_WAITS:
                    head, keep = waits[:-MAX_WAITS], waits[-MAX_WAITS:]
                    k = 0
                    while head:
                        chunk, head = head[:MAX_WAITS], head[MAX_WAITS:]
                        nop = mybir.InstNoOp(
                            name=f"{ins.name}-wsplit{k}", engine=ins.engine)
                        nop.sync_info = mybir.SyncInfo(on_wait=chunk, on_update=[])
                        out.append(nop)
                        k += 1
                    ins.sync_info = mybir.SyncInfo(
                        on_wait=keep, on_update=list(si.on_update))
                    changed = True
                out.append(ins)
            if changed:
                bb.instructions = out


def build_nc():
    nc = bass.Bass()
    x8_d = nc.declare_dram_parameter("x8", [98, NT8, B_LOC], FP8, isOutput=False)
    x16_d = nc.declare_dram_parameter("x16", [112, 7, B_LOC], FP16, isOutput=False)
    w8_d = nc.declare_dram_parameter("w8", [98, NT8, F1], FP8, isOutput=False)
    w16_d = nc.declare_dram_parameter("w16", [112, 7, F1], FP16, isOutput=False)
    w2_d = nc.declare_dram_parameter("w2dr", [128, 2, F2], FP8, isOutput=False)
    w3_d = nc.declare_dram_parameter("w3dr", [64, 2, F3], FP8, isOutput=False)
    w4_d = nc.declare_dram_parameter("w4T", [F3, F4], BF, isOutput=False)
    out_d = nc.declare_dram_parameter("out", [128, NCHUNK * 40], F32,
                                      isOutput=True)

    with tile.TileContext(nc) as tc:
        with tc.tile_pool(name="wpool", bufs=1) as wpool, \
             tc.tile_pool(name="x8pool", bufs=3) as x8pool, \
             tc.tile_pool(name="x16pool", bufs=3) as x16pool, \
             tc.tile_pool(name="apool", bufs=2) as apool, \
             tc.tile_pool(name="ps1", bufs=2, space="PSUM") as ps1, \
             tc.tile_pool(name="ps2", bufs=2, space="PSUM") as ps2, \
             tc.tile_pool(name="ps3", bufs=1, space="PSUM") as ps3, \
             tc.tile_pool(name="ps4", bufs=2, space="PSUM") as ps4:
            # weights for the first fp8 matmuls go first, then chunk-0 x in
            # quarter slices so the PE can start as soon as possible.
            w8t = wpool.tile([98, NT8, F1], FP8, name="w8t")
            nc.sync.dma_start(w8t[:], w8_d[:, :, :])
            x8t0 = x8pool.tile([98, NT8, NB], FP8, name="x8_0", tag="x8")
            for q in range(2):
                nc.sync.dma_start(x8t0[:, :, q * 256:(q + 1) * 256],
                                  x8_d[:, :, q * 256:(q + 1) * 256])
            w16t = wpool.tile([112, 7, F1], FP16, name="w16t")
            nc.scalar.dma_start(w16t[:], w16_d[:, :, :])
            x16t0 = x16pool.tile([112, 7, NB], FP16, name="x16_0", tag="x16")
            for q in range(2):
                nc.scalar.dma_start(x16t0[:, :, q * 256:(q + 1) * 256],
                                    x16_d[:, :, q * 256:(q + 1) * 256])
            w2t = wpool.tile([128, 2, F2], FP8, name="w2t")
            nc.scalar.dma_start(w2t[:], w2_d[:, :, :])
            w3t = wpool.tile([64, 2, F3], FP8, name="w3t")
            nc.scalar.dma_start(w3t[:], w3_d[:, :, :])
            w4t = wpool.tile([F3, F4], BF, name="w4t")
            nc.scalar.dma_start(w4t[:], w4_d[:, :])
            stage = wpool.tile([128, NCHUNK * 40], F32, name="stage")
            zb = wpool.tile([128, 1], F32, name="zb")
            nc.vector.memset(zb[:], 0.0)
            hb = wpool.tile([128, 1], F32, name="hb")
            nc.vector.memset(hb[:], 0.5)

            for c in range(NCHUNK):
                b0 = c * NB
                if c == 0:
                    x8t, x16t = x8t0, x16t0
                else:
                    x8t = x8pool.tile([98, NT8, NB], FP8, name=f"x8_{c}",
                                      tag="x8")
                    nc.sync.dma_start(x8t[:], x8_d[:, :, b0:b0 + NB])
                    x16t = x16pool.tile([112, 7, NB], FP16, name=f"x16_{c}",
                                        tag="x16")
                    nc.scalar.dma_start(x16t[:], x16_d[:, :, b0:b0 + NB])

                a1t = apool.tile([128, 2, NB], FP8, name=f"a1_{c}", tag="a1")
                for f in range(2):
                    p1 = ps1.tile([128, NB], F32, name=f"p1_{c}_{f}", tag="p1")
                    fs = slice(f * 128, (f + 1) * 128)
                    for j in range(2):
                        js = slice(j * 256, (j + 1) * 256)
                        for u in range(NT8 // 2):
                            nc.tensor.matmul(p1[:, js],
                                             w8t[:, 2 * u:2 * u + 2, fs],
                                             x8t[:, 2 * u:2 * u + 2, js],
                                             start=(u == 0), stop=False,
                                             perf_mode=DR)
                        for i in range(7):
                            nc.tensor.matmul(p1[:, js], w16t[:, i, fs],
                                             x16t[:, i, js],
                                             start=False, stop=(i == 6))
                    nc.scalar.activation(a1t[:, f, :], p1[:], AF.Sign,
                                         bias=zb[:], scale=1.0)

                p2 = ps2.tile([F2, NB], F32, name=f"p2_{c}", tag="p2")
                for j in range(2):
                    js = slice(j * 256, (j + 1) * 256)
                    nc.tensor.matmul(p2[:, js], w2t[:], a1t[:, :, js],
                                     start=True, stop=True, perf_mode=DR)
                a2t = apool.tile([64, 2, NB], FP8, name=f"a2_{c}", tag="a2")
                for s in range(2):
                    nc.scalar.activation(a2t[:, s, :],
                                         p2[s * 64:(s + 1) * 64, :],
                                         AF.Sign, bias=hb[:64], scale=1.0)

                p3 = ps3.tile([F3, NB], F32, name=f"p3_{c}", tag="p3")
                for j in range(2):
                    js = slice(j * 256, (j + 1) * 256)
                    nc.tensor.matmul(p3[:, js], w3t[:], a2t[:, :, js],
                                     start=True, stop=True, perf_mode=DR)
                a3t = apool.tile([F3, NB], BF, name=f"a3_{c}", tag="a3")
                nc.scalar.activation(a3t[:], p3[:], AF.Sign, bias=hb[:F3],
                                     scale=1.0)

                for sub in range(4):
                    p4 = ps4.tile([128, F4], F32, name=f"p4_{c}_{sub}",
                                  tag="p4")
                    nc.tensor.matmul(p4[:],
                                     a3t[:, sub * 128:(sub + 1) * 128],
                                     w4t[:], start=True, stop=True)
                    nc.vector.tensor_copy(
                        stage[:, c * 40 + sub * 10:c * 40 + (sub + 1) * 10],
                        p4[:])
                if c % 4 == 3:
                    cs = slice((c - 3) * 40, (c + 1) * 40)
                    nc.sync.dma_start(out_d[:, cs], stage[:, cs])
    fix_sync_waits(nc)
    return nc


def _sg(w):
    return np.where(np.asarray(w) >= 0, np.float32(1.0), np.float32(-1.0))


_NC_CACHE = {}


def kernel(x, w1, w2, w3, w4):
    if "nc" not in _NC_CACHE:
        _NC_CACHE["nc"] = build_nc()
    nc = _NC_CACHE["nc"]

    x = np.ascontiguousarray(np.asarray(x).reshape(-1, 784), dtype=np.float32)
    B = x.shape[0]
    w1sT = _sg(w1).T                        # [784, 256]

    # fp8 components of x (pass scales fold into weight rows) + fp16 residual
    xT = np.ascontiguousarray(x.T)          # [784, B]
    comps8 = []
    w8rows = []
    rem = xT
    scale = np.float32(1.0)
    for p in range(N8PASS):
        q = (rem * scale).astype(E4M3)
        comps8.append(q)
        w8rows.append(w1sT / scale)
        rem = rem - q.astype(np.float32) / scale
        scale = np.float32(scale * 16.0)
    x8 = np.stack(comps8, axis=0)           # [N8PASS, 784, B] e4m3
    x8 = np.ascontiguousarray(
        x8.reshape(N8PASS, 8, 98, B).transpose(2, 0, 1, 3).reshape(98, NT8, B))
    w8 = np.stack(w8rows, axis=0)           # [N8PASS, 784, 256]
    w8 = np.ascontiguousarray(
        w8.reshape(N8PASS, 8, 98, F1).transpose(2, 0, 1, 3)
        .reshape(98, NT8, F1)).astype(E4M3)
    x16 = np.ascontiguousarray(
        rem.astype(np.float16).reshape(7, 112, B).transpose(1, 0, 2))
    w16 = np.ascontiguousarray(
        w1sT.reshape(7, 112, F1).transpose(1, 0, 2)).astype(np.float16)

    w2sT = _sg(w2).T                        # [256, 128]
    w2dr = np.ascontiguousarray(
        w2sT.reshape(2, 128, F2).transpose(1, 0, 2)).astype(E4M3)
    w3sT = _sg(w3).T                        # [128, 32]
    w3dr = np.ascontiguousarray(
        w3sT.reshape(2, 64, F3).transpose(1, 0, 2)).astype(E4M3)
    w4T = np.ascontiguousarray(_sg(w4).T).astype(BF16)   # [32, 10]

    wm = {"w8": w8, "w16": w16, "w2dr": w2dr, "w3dr": w3dr, "w4T": w4T}
    maps = []
    for core in range(N_CORES):
        m = dict(wm)
        bs = slice(core * B_LOC, (core + 1) * B_LOC)
        m["x8"] = np.ascontiguousarray(x8[:, :, bs])
        m["x16"] = np.ascontiguousarray(x16[:, :, bs])
        maps.append(m)

    res = None
    last_exc = None
    for attempt in range(3):
        try:
            res = run_bass_kernel_spmd(nc, maps, list(range(N_CORES)))
            break
        except Exception as e:  # transient NRT/device errors: retry
            last_exc = e
            import time
            time.sleep(5 * (attempt + 1))
    if res is None:
        raise last_exc
    # stage layout: [p, c*40 + sub*10 + f]  <->  batch b = c*512 + sub*128 + p
    outs = []
    for r in res.results:
        o = r["out"].reshape(128, NCHUNK, 4, F4)
        outs.append(np.ascontiguousarray(
            o.transpose(1, 2, 0, 3).reshape(B_LOC, F4)))
    return np.ascontiguousarray(np.concatenate(outs, axis=0)).astype(np.float32)


# revision 16
# speedup vs baseline: 1.3692x; 1.0002x over previous
"""Trainium2 Bass kernel: binarized-MLP forward (784-256-128-32-10, ste_sign).

Strategy
--------
Pure data parallel over 8 NeuronCores: batch 65536 -> 8 shards of 8192 rows;
tiny sign-binarized weights replicated (binarized + packed on the host). Each
core runs the full 4-layer net on its shard feature-major ([features, batch]
tiles, batch as the moving dim) so layer N's output feeds layer N+1 with no
transposes. Outputs leave batch-major via a flipped final matmul.

Layer 1 (the only real-valued matmul; everything downstream only sees
sign(h1)) uses a multi-component split of x whose per-pass scales are folded
into the replicated weight k-rows, so all passes accumulate into ONE PSUM
group with no combine step:

  N8PASS=1: x ~= e4m3(x) + fp16(r1)            (3 B/elem, err ~2^-16|x|)
  N8PASS=2: x ~= e4m3(x) + e4m3(r1*16)/16 + fp16(r2)   (4 B/elem, ~2^-20|x|)

fp8 passes run as DoubleRow matmuls (2 k-tiles per instruction, 0.5
cycles/col -> 4x bf16 throughput); 784 = 98*8 for the fp8 part (4 DoubleRow
pairs per pass, no tail) and 112*7 for the fp16 part (7 full matmuls, no
tail). Layers 2-3 have +-1 inputs and +-1 weights, exact in fp8: activations
are written with feature-halves as DoubleRow slots ([128,2,512] / [64,2,512])
so L2/L3 are single DoubleRow units. L4 is flipped (lhsT = a3 slices, rhs =
w4^T moving, 10-wide): 10 cycles per matmul and batch-major [128,10] PSUM
output, staged into one [128, 640] SBUF tile and DMA'd out in 4 bulk
transfers (partition-major, 2560 B contiguous per partition).

HW Sign(0) = 0, so the integer-valued pre-activations (layers 2,3) use
Sign(h + 0.5), which reproduces the reference's sign(0)=+1 exactly.

This walrus build rejects instructions carrying more than one semaphore wait
("Too many sync wait commands"), so after Tile scheduling, excess waits are
split onto preceding same-engine NoOps (fix_sync_waits).
"""
import sys
sys.path.insert(0, '/opt/trn_rl_repo')
import numpy as np
import ml_dtypes
import concourse.bass as bass
import concourse.mybir as mybir
from concourse import tile
from concourse.bass_utils import run_bass_kernel_spmd

BF16 = ml_dtypes.bfloat16
E4M3 = ml_dtypes.float8_e4m3
F32 = mybir.dt.float32
FP16 = mybir.dt.float16
FP8 = mybir.dt.float8e4
BF = mybir.dt.bfloat16
AF = mybir.ActivationFunctionType
DR = mybir.MatmulPerfMode.DoubleRow

N_CORES = 8
B_LOC = 8192          # batch rows per core
NB = 512              # batch columns per compute chunk (one fp32 PSUM bank)
NCHUNK = B_LOC // NB
N8PASS = 1            # fp8 components of x (1 => +fp16 residual = 3 B/elem)
NT8 = 8 * N8PASS      # fp8 k-tiles of 98 rows
F1, F2, F3, F4 = 256, 128, 32, 10
MAX_WAITS = 1


def fix_sync_waits(nc):
    for fn in nc.m.functions:
        for bb in fn.blocks:
            out = []
            changed = False
            for ins in bb.instructions:
                si = ins.sync_info
                waits = list(si.on_wait) if si is not None else []
                if len(waits) > MAX_WAITS:
                    head, keep = waits[:-MAX_WAITS], waits[-MAX_WAITS:]
                    k = 0
                    while head:
                        chunk, head = head[:MAX_WAITS], head[MAX_WAITS:]
                        nop = mybir.InstNoOp(
                            name=f"{ins.name}-wsplit{k}", engine=ins.engine)
                        nop.sync_info = mybir.SyncInfo(on_wait=chunk, on_update=[])
                        out.append(nop)
                        k += 1
                    ins.sync_info = mybir.SyncInfo(
                        on_wait=keep, on_update=list(si.on_update))
                    changed = True
                out.append(ins)
            if changed:
                bb.instructions = out


def build_nc():
    nc = bass.Bass()
    x8_d = nc.declare_dram_parameter("x8", [98, NT8, B_LOC], FP8, isOutput=False)
    x16_d = nc.declare_dram_parameter("x16", [112, 7, B_LOC], FP16, isOutput=False)
    w8_d = nc.declare_dram_parameter("w8", [98, NT8, F1], FP8, isOutput=False)
    w16_d = nc.declare_dram_parameter("w16", [112, 7, F1], FP16, isOutput=False)
    w2_d = nc.declare_dram_parameter("w2dr", [128, 2, F2], FP8, isOutput=False)
    w3_d = nc.declare_dram_parameter("w3dr", [64, 2, F3], FP8, isOutput=False)
    w4_d = nc.declare_dram_parameter("w4T", [F3, F4], BF, isOutput=False)
    out_d = nc.declare_dram_parameter("out", [128, NCHUNK * 40], F32,
                                      isOutput=True)

    with tile.TileContext(nc) as tc:
        with tc.tile_pool(name="wpool", bufs=1) as wpool, \
             tc.tile_pool(name="x8pool", bufs=3) as x8pool, \
             tc.tile_pool(name="x16pool", bufs=3) as x16pool, \
             tc.tile_pool(name="apool", bufs=3) as apool, \
             tc.tile_pool(name="ps1", bufs=3, space="PSUM") as ps1, \
             tc.tile_pool(name="ps2", bufs=2, space="PSUM") as ps2, \
             tc.tile_pool(name="ps3", bufs=1, space="PSUM") as ps3, \
             tc.tile_pool(name="ps4", bufs=2, space="PSUM") as ps4:
            # head: land the first DR matmul's operands (w8 pair 0, x8 pair 0)
            # before anything else, split across both HWDGE queues; the fp16
            # pass's operands follow, then the small weights.
            w8t = wpool.tile([98, NT8, F1], FP8, name="w8t")
            nc.sync.dma_start(w8t[:, 0:2, :], w8_d[:, 0:2, :])
            x8t0 = x8pool.tile([98, NT8, NB], FP8, name="x8_0", tag="x8")
            nc.scalar.dma_start(x8t0[:, 0:2, :], x8_d[:, 0:2, :NB])
            nc.sync.dma_start(w8t[:, 2:NT8, :], w8_d[:, 2:NT8, :])
            nc.scalar.dma_start(x8t0[:, 2:NT8, :], x8_d[:, 2:NT8, :NB])
            x16t0 = x16pool.tile([112, 7, NB], FP16, name="x16_0", tag="x16")
            nc.sync.dma_start(x16t0[:, 0:4, :], x16_d[:, 0:4, :NB])
            w16t = wpool.tile([112, 7, F1], FP16, name="w16t")
            nc.scalar.dma_start(w16t[:], w16_d[:, :, :])
            nc.sync.dma_start(x16t0[:, 4:7, :], x16_d[:, 4:7, :NB])
            w2t = wpool.tile([128, 2, F2], FP8, name="w2t")
            nc.scalar.dma_start(w2t[:], w2_d[:, :, :])
            w3t = wpool.tile([64, 2, F3], FP8, name="w3t")
            nc.scalar.dma_start(w3t[:], w3_d[:, :, :])
            w4t = wpool.tile([F3, F4], BF, name="w4t")
            nc.scalar.dma_start(w4t[:], w4_d[:, :])
            stage = wpool.tile([128, NCHUNK * 40], F32, name="stage")
            zb = wpool.tile([128, 1], F32, name="zb")
            nc.vector.memset(zb[:], 0.0)
            hb = wpool.tile([128, 1], F32, name="hb")
            nc.vector.memset(hb[:], 0.5)

            for c in range(NCHUNK):
                b0 = c * NB
                if c == 0:
                    x8t, x16t = x8t0, x16t0
                else:
                    x8t = x8pool.tile([98, NT8, NB], FP8, name=f"x8_{c}",
                                      tag="x8")
                    nc.sync.dma_start(x8t[:], x8_d[:, :, b0:b0 + NB])
                    x16t = x16pool.tile([112, 7, NB], FP16, name=f"x16_{c}",
                                        tag="x16")
                    nc.scalar.dma_start(x16t[:], x16_d[:, :, b0:b0 + NB])

                a1t = apool.tile([128, 2, NB], FP8, name=f"a1_{c}", tag="a1")
                for f in range(2):
                    p1 = ps1.tile([128, NB], F32, name=f"p1_{c}_{f}", tag="p1")
                    fs = slice(f * 128, (f + 1) * 128)
                    for j in range(2):
                        js = slice(j * 256, (j + 1) * 256)
                        for u in range(NT8 // 2):
                            nc.tensor.matmul(p1[:, js],
                                             w8t[:, 2 * u:2 * u + 2, fs],
                                             x8t[:, 2 * u:2 * u + 2, js],
                                             start=(u == 0), stop=False,
                                             perf_mode=DR)
                        for i in range(7):
                            nc.tensor.matmul(p1[:, js], w16t[:, i, fs],
                                             x16t[:, i, js],
                                             start=False, stop=(i == 6))
                    nc.scalar.activation(a1t[:, f, :], p1[:], AF.Sign,
                                         bias=zb[:], scale=1.0)

                p2 = ps2.tile([F2, NB], F32, name=f"p2_{c}", tag="p2")
                for j in range(2):
                    js = slice(j * 256, (j + 1) * 256)
                    nc.tensor.matmul(p2[:, js], w2t[:], a1t[:, :, js],
                                     start=True, stop=True, perf_mode=DR)
                a2t = apool.tile([64, 2, NB], FP8, name=f"a2_{c}", tag="a2")
                for s in range(2):
                    nc.scalar.activation(a2t[:, s, :],
                                         p2[s * 64:(s + 1) * 64, :],
                                         AF.Sign, bias=hb[:64], scale=1.0)

                p3 = ps3.tile([F3, NB], F32, name=f"p3_{c}", tag="p3")
                for j in range(2):
                    js = slice(j * 256, (j + 1) * 256)
                    nc.tensor.matmul(p3[:, js], w3t[:], a2t[:, :, js],
                                     start=True, stop=True, perf_mode=DR)
                a3t = apool.tile([F3, NB], BF, name=f"a3_{c}", tag="a3")
                nc.scalar.activation(a3t[:], p3[:], AF.Sign, bias=hb[:F3],
                                     scale=1.0)

                for sub in range(4):
                    p4 = ps4.tile([128, F4], F32, name=f"p4_{c}_{sub}",
                                  tag="p4")
                    nc.tensor.matmul(p4[:],
                                     a3t[:, sub * 128:(sub + 1) * 128],
                                     w4t[:], start=True, stop=True)
                    nc.vector.tensor_copy(
                        stage[:, c * 40 + sub * 10:c * 40 + (sub + 1) * 10],
                        p4[:])
                if c % 4 == 3:
                    cs = slice((c - 3) * 40, (c + 1) * 40)
                    nc.gpsimd.dma_start(out_d[:, cs], stage[:, cs])
    fix_sync_waits(nc)
    return nc


def _sg(w):
    return np.where(np.asarray(w) >= 0, np.float32(1.0), np.float32(-1.0))


_NC_CACHE = {}


def kernel(x, w1, w2, w3, w4):
    if "nc" not in _NC_CACHE:
        _NC_CACHE["nc"] = build_nc()
    nc = _NC_CACHE["nc"]

    x = np.ascontiguousarray(np.asarray(x).reshape(-1, 784), dtype=np.float32)
    B = x.shape[0]
    w1sT = _sg(w1).T                        # [784, 256]

    # fp8 components of x (pass scales fold into weight rows) + fp16 residual
    xT = np.ascontiguousarray(x.T)          # [784, B]
    comps8 = []
    w8rows = []
    rem = xT
    scale = np.float32(1.0)
    for p in range(N8PASS):
        q = (rem * scale).astype(E4M3)
        comps8.append(q)
        w8rows.append(w1sT / scale)
        rem = rem - q.astype(np.float32) / scale
        scale = np.float32(scale * 16.0)
    x8 = np.stack(comps8, axis=0)           # [N8PASS, 784, B] e4m3
    x8 = np.ascontiguousarray(
        x8.reshape(N8PASS, 8, 98, B).transpose(2, 0, 1, 3).reshape(98, NT8, B))
    w8 = np.stack(w8rows, axis=0)           # [N8PASS, 784, 256]
    w8 = np.ascontiguousarray(
        w8.reshape(N8PASS, 8, 98, F1).transpose(2, 0, 1, 3)
        .reshape(98, NT8, F1)).astype(E4M3)
    x16 = np.ascontiguousarray(
        rem.astype(np.float16).reshape(7, 112, B).transpose(1, 0, 2))
    w16 = np.ascontiguousarray(
        w1sT.reshape(7, 112, F1).transpose(1, 0, 2)).astype(np.float16)

    w2sT = _sg(w2).T                        # [256, 128]
    w2dr = np.ascontiguousarray(
        w2sT.reshape(2, 128, F2).transpose(1, 0, 2)).astype(E4M3)
    w3sT = _sg(w3).T                        # [128, 32]
    w3dr = np.ascontiguousarray(
        w3sT.reshape(2, 64, F3).transpose(1, 0, 2)).astype(E4M3)
    w4T = np.ascontiguousarray(_sg(w4).T).astype(BF16)   # [32, 10]

    wm = {"w8": w8, "w16": w16, "w2dr": w2dr, "w3dr": w3dr, "w4T": w4T}
    maps = []
    for core in range(N_CORES):
        m = dict(wm)
        bs = slice(core * B_LOC, (core + 1) * B_LOC)
        m["x8"] = np.ascontiguousarray(x8[:, :, bs])
        m["x16"] = np.ascontiguousarray(x16[:, :, bs])
        maps.append(m)

    res = None
    last_exc = None
    for attempt in range(3):
        try:
            res = run_bass_kernel_spmd(nc, maps, list(range(N_CORES)))
            break
        except Exception as e:  # transient NRT/device errors: retry
            last_exc = e
            import time
            time.sleep(5 * (attempt + 1))
    if res is None:
        raise last_exc
    # stage layout: [p, c*40 + sub*10 + f]  <->  batch b = c*512 + sub*128 + p
    outs = []
    for r in res.results:
        o = r["out"].reshape(128, NCHUNK, 4, F4)
        outs.append(np.ascontiguousarray(
            o.transpose(1, 2, 0, 3).reshape(B_LOC, F4)))
    return np.ascontiguousarray(np.concatenate(outs, axis=0)).astype(np.float32)


# revision 18
# speedup vs baseline: 1.3776x; 1.0061x over previous
"""Trainium2 Bass kernel: binarized-MLP forward (784-256-128-32-10, ste_sign).

Strategy
--------
Pure data parallel over 8 NeuronCores: batch 65536 -> 8 shards of 8192 rows;
tiny sign-binarized weights replicated (binarized + packed on the host). Each
core runs the full 4-layer net on its shard feature-major ([features, batch]
tiles, batch as the moving dim) so layer N's output feeds layer N+1 with no
transposes. Outputs leave batch-major via a flipped final matmul.

Layer 1 (the only real-valued matmul; everything downstream only sees
sign(h1)) uses a multi-component split of x whose per-pass scales are folded
into the replicated weight k-rows, so all passes accumulate into ONE PSUM
group with no combine step:

  N8PASS=1: x ~= e4m3(x) + fp16(r1)            (3 B/elem, err ~2^-16|x|)
  N8PASS=2: x ~= e4m3(x) + e4m3(r1*16)/16 + fp16(r2)   (4 B/elem, ~2^-20|x|)

fp8 passes run as DoubleRow matmuls (2 k-tiles per instruction, 0.5
cycles/col -> 4x bf16 throughput); 784 = 98*8 for the fp8 part (4 DoubleRow
pairs per pass, no tail) and 112*7 for the fp16 part (7 full matmuls, no
tail). Layers 2-3 have +-1 inputs and +-1 weights, exact in fp8: activations
are written with feature-halves as DoubleRow slots ([128,2,512] / [64,2,512])
so L2/L3 are single DoubleRow units. L4 is flipped (lhsT = a3 slices, rhs =
w4^T moving, 10-wide): 10 cycles per matmul and batch-major [128,10] PSUM
output, staged into one [128, 640] SBUF tile and DMA'd out in 4 bulk
transfers (partition-major, 2560 B contiguous per partition).

HW Sign(0) = 0, so the integer-valued pre-activations (layers 2,3) use
Sign(h + 0.5), which reproduces the reference's sign(0)=+1 exactly.

This walrus build rejects instructions carrying more than one semaphore wait
("Too many sync wait commands"), so after Tile scheduling, excess waits are
split onto preceding same-engine NoOps (fix_sync_waits).
"""
import sys
sys.path.insert(0, '/opt/trn_rl_repo')
import numpy as np
import ml_dtypes
import concourse.bass as bass
import concourse.mybir as mybir
from concourse import tile
from concourse.bass_utils import run_bass_kernel_spmd

BF16 = ml_dtypes.bfloat16
E4M3 = ml_dtypes.float8_e4m3
F32 = mybir.dt.float32
FP16 = mybir.dt.float16
FP8 = mybir.dt.float8e4
BF = mybir.dt.bfloat16
AF = mybir.ActivationFunctionType
DR = mybir.MatmulPerfMode.DoubleRow

N_CORES = 8
B_LOC = 8192          # batch rows per core
NB = 512              # batch columns per compute chunk (one fp32 PSUM bank)
NCHUNK = B_LOC // NB
N8PASS = 1            # fp8 components of x (1 => +fp16 residual = 3 B/elem)
NT8 = 8 * N8PASS      # fp8 k-tiles of 98 rows
F1, F2, F3, F4 = 256, 128, 32, 10
MAX_WAITS = 1


def fix_sync_waits(nc):
    for fn in nc.m.functions:
        for bb in fn.blocks:
            out = []
            changed = False
            for ins in bb.instructions:
                si = ins.sync_info
                waits = list(si.on_wait) if si is not None else []
                if len(waits) > MAX_WAITS:
                    head, keep = waits[:-MAX_WAITS], waits[-MAX_WAITS:]
                    k = 0
                    while head:
                        chunk, head = head[:MAX_WAITS], head[MAX_WAITS:]
                        nop = mybir.InstNoOp(
                            name=f"{ins.name}-wsplit{k}", engine=ins.engine)
                        nop.sync_info = mybir.SyncInfo(on_wait=chunk, on_update=[])
                        out.append(nop)
                        k += 1
                    ins.sync_info = mybir.SyncInfo(
                        on_wait=keep, on_update=list(si.on_update))
                    changed = True
                out.append(ins)
            if changed:
                bb.instructions = out


def build_nc():
    nc = bass.Bass()
    x8_d = nc.declare_dram_parameter("x8", [98, NT8, B_LOC], FP8, isOutput=False)
    x16_d = nc.declare_dram_parameter("x16", [112, 7, B_LOC], FP16, isOutput=False)
    w8_d = nc.declare_dram_parameter("w8", [98, NT8, F1], FP8, isOutput=False)
    w16_d = nc.declare_dram_parameter("w16", [112, 7, F1], FP16, isOutput=False)
    w2_d = nc.declare_dram_parameter("w2dr", [128, 2, F2], FP8, isOutput=False)
    w3_d = nc.declare_dram_parameter("w3dr", [64, 2, F3], FP8, isOutput=False)
    w4_d = nc.declare_dram_parameter("w4T", [F3, F4], BF, isOutput=False)
    out_d = nc.declare_dram_parameter("out", [128, NCHUNK * 40], F32,
                                      isOutput=True)

    with tile.TileContext(nc) as tc:
        with tc.tile_pool(name="wpool", bufs=1) as wpool, \
             tc.tile_pool(name="x8pool", bufs=4) as x8pool, \
             tc.tile_pool(name="x16pool", bufs=4) as x16pool, \
             tc.tile_pool(name="apool", bufs=3) as apool, \
             tc.tile_pool(name="ps1", bufs=2, space="PSUM") as ps1, \
             tc.tile_pool(name="ps2", bufs=1, space="PSUM") as ps2, \
             tc.tile_pool(name="ps3", bufs=1, space="PSUM") as ps3, \
             tc.tile_pool(name="ps4", bufs=4, space="PSUM") as ps4:
            # head: land the first DR matmul's operands (w8 pair 0, x8 pair 0)
            # before anything else, split across both HWDGE queues; the fp16
            # pass's operands follow, then the small weights.
            w8t = wpool.tile([98, NT8, F1], FP8, name="w8t")
            nc.sync.dma_start(w8t[:, 0:2, :], w8_d[:, 0:2, :])
            x8t0 = x8pool.tile([98, NT8, NB], FP8, name="x8_0", tag="x8")
            nc.scalar.dma_start(x8t0[:, 0:2, :], x8_d[:, 0:2, :NB])
            nc.sync.dma_start(w8t[:, 2:NT8, :], w8_d[:, 2:NT8, :])
            nc.scalar.dma_start(x8t0[:, 2:NT8, :], x8_d[:, 2:NT8, :NB])
            w16t = wpool.tile([112, 7, F1], FP16, name="w16t")
            x16t0 = x16pool.tile([112, 7, NB], FP16, name="x16_0", tag="x16")
            nc.sync.dma_start(w16t[:, 0:2, :], w16_d[:, 0:2, :])
            nc.scalar.dma_start(x16t0[:, 0:2, :], x16_d[:, 0:2, :NB])
            nc.sync.dma_start(w16t[:, 2:7, :], w16_d[:, 2:7, :])
            nc.scalar.dma_start(x16t0[:, 2:7, :], x16_d[:, 2:7, :NB])
            w2t = wpool.tile([128, 2, F2], FP8, name="w2t")
            nc.scalar.dma_start(w2t[:], w2_d[:, :, :])
            w3t = wpool.tile([64, 2, F3], FP8, name="w3t")
            nc.scalar.dma_start(w3t[:], w3_d[:, :, :])
            w4t = wpool.tile([F3, F4], BF, name="w4t")
            nc.scalar.dma_start(w4t[:], w4_d[:, :])
            stage = wpool.tile([128, NCHUNK * 40], F32, name="stage")
            zb = wpool.tile([128, 1], F32, name="zb")
            nc.vector.memset(zb[:], 0.0)
            hb = wpool.tile([128, 1], F32, name="hb")
            nc.vector.memset(hb[:], 0.5)

            for c in range(NCHUNK):
                b0 = c * NB
                if c == 0:
                    x8t, x16t = x8t0, x16t0
                else:
                    x8t = x8pool.tile([98, NT8, NB], FP8, name=f"x8_{c}",
                                      tag="x8")
                    nc.sync.dma_start(x8t[:], x8_d[:, :, b0:b0 + NB])
                    x16t = x16pool.tile([112, 7, NB], FP16, name=f"x16_{c}",
                                        tag="x16")
                    nc.scalar.dma_start(x16t[:], x16_d[:, :, b0:b0 + NB])

                a1t = apool.tile([128, 2, NB], FP8, name=f"a1_{c}", tag="a1")
                for f in range(2):
                    p1 = ps1.tile([128, NB], F32, name=f"p1_{c}_{f}", tag="p1")
                    fs = slice(f * 128, (f + 1) * 128)
                    for j in range(2):
                        js = slice(j * 256, (j + 1) * 256)
                        for u in range(NT8 // 2):
                            nc.tensor.matmul(p1[:, js],
                                             w8t[:, 2 * u:2 * u + 2, fs],
                                             x8t[:, 2 * u:2 * u + 2, js],
                                             start=(u == 0), stop=False,
                                             perf_mode=DR)
                        for i in range(7):
                            nc.tensor.matmul(p1[:, js], w16t[:, i, fs],
                                             x16t[:, i, js],
                                             start=False, stop=(i == 6))
                    nc.scalar.activation(a1t[:, f, :], p1[:], AF.Sign,
                                         bias=zb[:], scale=1.0)

                p2 = ps2.tile([F2, NB], F32, name=f"p2_{c}", tag="p2")
                for j in range(2):
                    js = slice(j * 256, (j + 1) * 256)
                    nc.tensor.matmul(p2[:, js], w2t[:], a1t[:, :, js],
                                     start=True, stop=True, perf_mode=DR)
                a2t = apool.tile([64, 2, NB], FP8, name=f"a2_{c}", tag="a2")
                for s in range(2):
                    nc.scalar.activation(a2t[:, s, :],
                                         p2[s * 64:(s + 1) * 64, :],
                                         AF.Sign, bias=hb[:64], scale=1.0)

                p3 = ps3.tile([F3, NB], F32, name=f"p3_{c}", tag="p3")
                for j in range(2):
                    js = slice(j * 256, (j + 1) * 256)
                    nc.tensor.matmul(p3[:, js], w3t[:], a2t[:, :, js],
                                     start=True, stop=True, perf_mode=DR)
                a3t = apool.tile([F3, NB], BF, name=f"a3_{c}", tag="a3")
                nc.scalar.activation(a3t[:], p3[:], AF.Sign, bias=hb[:F3],
                                     scale=1.0)

                for sub in range(4):
                    p4 = ps4.tile([128, F4], F32, name=f"p4_{c}_{sub}",
                                  tag="p4")
                    nc.tensor.matmul(p4[:],
                                     a3t[:, sub * 128:(sub + 1) * 128],
                                     w4t[:], start=True, stop=True)
                    nc.vector.tensor_copy(
                        stage[:, c * 40 + sub * 10:c * 40 + (sub + 1) * 10],
                        p4[:])
                if c % 4 == 3:
                    cs = slice((c - 3) * 40, (c + 1) * 40)
                    nc.gpsimd.dma_start(out_d[:, cs], stage[:, cs])
    fix_sync_waits(nc)
    return nc


def _sg(w):
    return np.where(np.asarray(w) >= 0, np.float32(1.0), np.float32(-1.0))


_NC_CACHE = {}


def kernel(x, w1, w2, w3, w4):
    if "nc" not in _NC_CACHE:
        _NC_CACHE["nc"] = build_nc()
    nc = _NC_CACHE["nc"]

    x = np.ascontiguousarray(np.asarray(x).reshape(-1, 784), dtype=np.float32)
    B = x.shape[0]
    w1sT = _sg(w1).T                        # [784, 256]

    # fp8 components of x (pass scales fold into weight rows) + fp16 residual
    xT = np.ascontiguousarray(x.T)          # [784, B]
    comps8 = []
    w8rows = []
    rem = xT
    scale = np.float32(1.0)
    for p in range(N8PASS):
        q = (rem * scale).astype(E4M3)
        comps8.append(q)
        w8rows.append(w1sT / scale)
        rem = rem - q.astype(np.float32) / scale
        scale = np.float32(scale * 16.0)
    x8 = np.stack(comps8, axis=0)           # [N8PASS, 784, B] e4m3
    x8 = np.ascontiguousarray(
        x8.reshape(N8PASS, 8, 98, B).transpose(2, 0, 1, 3).reshape(98, NT8, B))
    w8 = np.stack(w8rows, axis=0)           # [N8PASS, 784, 256]
    w8 = np.ascontiguousarray(
        w8.reshape(N8PASS, 8, 98, F1).transpose(2, 0, 1, 3)
        .reshape(98, NT8, F1)).astype(E4M3)
    x16 = np.ascontiguousarray(
        rem.astype(np.float16).reshape(7, 112, B).transpose(1, 0, 2))
    w16 = np.ascontiguousarray(
        w1sT.reshape(7, 112, F1).transpose(1, 0, 2)).astype(np.float16)

    w2sT = _sg(w2).T                        # [256, 128]
    w2dr = np.ascontiguousarray(
        w2sT.reshape(2, 128, F2).transpose(1, 0, 2)).astype(E4M3)
    w3sT = _sg(w3).T                        # [128, 32]
    w3dr = np.ascontiguousarray(
        w3sT.reshape(2, 64, F3).transpose(1, 0, 2)).astype(E4M3)
    w4T = np.ascontiguousarray(_sg(w4).T).astype(BF16)   # [32, 10]

    wm = {"w8": w8, "w16": w16, "w2dr": w2dr, "w3dr": w3dr, "w4T": w4T}
    maps = []
    for core in range(N_CORES):
        m = dict(wm)
        bs = slice(core * B_LOC, (core + 1) * B_LOC)
        m["x8"] = np.ascontiguousarray(x8[:, :, bs])
        m["x16"] = np.ascontiguousarray(x16[:, :, bs])
        maps.append(m)

    res = None
    last_exc = None
    for attempt in range(3):
        try:
            res = run_bass_kernel_spmd(nc, maps, list(range(N_CORES)))
            break
        except Exception as e:  # transient NRT/device errors: retry
            last_exc = e
            import time
            time.sleep(5 * (attempt + 1))
    if res is None:
        raise last_exc
    # stage layout: [p, c*40 + sub*10 + f]  <->  batch b = c*512 + sub*128 + p
    outs = []
    for r in res.results:
        o = r["out"].reshape(128, NCHUNK, 4, F4)
        outs.append(np.ascontiguousarray(
            o.transpose(1, 2, 0, 3).reshape(B_LOC, F4)))
    return np.ascontiguousarray(np.concatenate(outs, axis=0)).astype(np.float32)
